# revision 29
# baseline (speedup 1.0000x reference)
"""Multi-head attention (no softmax) on 8 trn2 NeuronCores.

Reference: out = ((x @ Wqkv.T -> q,k,v per head) ; (q @ k.T * s) @ v ; concat ; @ Wproj.T)

Because there is no softmax the attention is linear:
    (q @ k.T) @ v == q @ (k.T @ v),  k.T @ v is only 64x64 per head,
so the T x T score matrices never need to exist. Per head:
    M_h = (s * k_h).T @ v_h        (64 x 64, reduced over ALL tokens of the batch)
    out += (q_h @ M_h) @ Wproj_h.T

Sharding: token-parallel. Core c owns batch b=c//2, token half c%2 (512 tokens).
M_h needs a reduction over the full batch -> pairwise AllReduce(add) between
the two cores of each batch, split into TWO 64KB ops (heads 0-7 after the
first kv half, heads 8-15 after the second) so the first returns early.

Everything runs in bf16 (same PE rate as fp32r, half the HBM traffic; rel err
~5e-3 vs the 2e-2 gate). PSUM accumulates fp32. The 1/8 head scale is folded
into W_k on the host (exact).

Schedule (the point of this revision): the PE never waits on the collective
with real work available. Order: kv half0 -> M0 -> AR0 trigger -> kv half1 ->
M1 -> AR1 trigger -> q (all 8 blocks) -> att heads0-7 (needs AR0) -> out
PHASE A: all 8 (oc,tt) groups accumulate contraction f=0..3 in PSUM and stash
the fp32 partial in SBUF -> att heads8-15 (needs AR1) -> out PHASE B:
accumulate f=4..7, add the stash back (DVE tensor_add), store. Phase A is
~8us of real matmul cover for AR1 latency/partner skew; the old 56-dummy
filler is gone. A dummy 256B AllReduce still pre-pays the one-time collective
setup during the fill (first op costs ~13us of semaphore hops).

DMAs are coarse - the host pre-swizzles every operand into its exact
[128, cols] SBUF layout so each logical group is ONE contiguous dma_start
(the Sync sequencer spends ~0.6us of issue time per dma_start). The fill is
split across two queues (sync: x + k1/v1 weights; gpsimd: k0/v0 weights
before the dummy trigger) so issue time halves and kv can start on the first
0.25MB. wq/wp (4MB) defer to the scalar queue mid-kv. Memsets ride the idle
vector engine at t=0 so warmup matmuls start immediately; output stores
round-robin 3 DMA queues.
"""

import numpy as np

B, T, E = 4, 1024, 1024
NH, HD = 16, 64
N_CORES = 8
TPC = T // 2  # tokens per core = 512

_built = None


def _build():
    """Build + compile the 8-core SPMD Bass program once."""
    global _built
    if _built is not None:
        return _built

    import concourse.mybir as mybir
    import concourse.tile as tile
    from concourse import bacc

    f32 = mybir.dt.float32
    bf16 = mybir.dt.bfloat16
    GROUPS = [[0, 1], [2, 3], [4, 5], [6, 7]]

    nc = bacc.Bacc("TRN2", target_bir_lowering=False, debug=False, num_devices=N_CORES)
    # x pre-swizzled: [128 part, e*512 + tok]
    xd = nc.dram_tensor("xd", [128, 4096], bf16, kind="ExternalInput").ap()
    # kv weights: 4 stream groups (k0, v0, k1, v1), each [128 part, e*512 + kvf]
    wkvd = nc.dram_tensor("wkvd", [4 * 128, 8 * 512], bf16, kind="ExternalInput").ap()
    # q weights: 2 column-half groups, each [128 part, e*512 + qf]
    wqd = nc.dram_tensor("wqd", [2 * 128, 8 * 512], bf16, kind="ExternalInput").ap()
    # proj weights, oc-major: [128 part, oc*4096 + f*512 + c]
    wpd = nc.dram_tensor("wpd", [128, 8 * 1024], bf16, kind="ExternalInput").ap()
    out = nc.dram_tensor("out", [TPC, E], f32, kind="ExternalOutput").ap()

    evict_i = [0]

    def evict(dst, src):
        # spread PSUM->SBUF eviction copies across DVE and ACT
        if evict_i[0] % 2 == 0:
            nc.vector.tensor_copy(dst, src)
        else:
            nc.scalar.copy(dst, src)
        evict_i[0] += 1

    with tile.TileContext(nc) as tc:
        with (
            tc.tile_pool(name="xp", bufs=1) as xp,
            tc.tile_pool(name="wkvp", bufs=1) as wkvp,
            tc.tile_pool(name="kvp", bufs=1) as kvp,
            tc.tile_pool(name="wqp", bufs=1) as wqp,
            tc.tile_pool(name="wpp", bufs=1) as wpp,
            tc.tile_pool(name="qp", bufs=1) as qp,
            tc.tile_pool(name="mres", bufs=1) as mres,
            tc.tile_pool(name="stp", bufs=1) as stp,
            tc.tile_pool(name="warm", bufs=1) as warmp,
            tc.tile_pool(name="dram", bufs=1, space="DRAM") as dram,
            tc.tile_pool(name="psA", bufs=6, space="PSUM") as psA,
            tc.tile_pool(name="psM", bufs=2, space="PSUM") as psM,
        ):
            xsb = xp.tile([128, 4096], bf16, tag="x")  # col = e*512 + tok
            wkv = [wkvp.tile([128, 4096], bf16, tag=f"wkv{s}", name=f"wkv{s}")
                   for s in range(4)]

            # ---- t=0: memsets on the idle DVE so the PE warms immediately ----
            warm = warmp.tile([128, 512], bf16, tag="warm")
            nc.vector.memset(warm[:].bitcast(f32), 0.0)
            # Msb holds both AR payloads in block-diagonal layout; the
            # off-diagonal zeros ride along so the AR result can return as a
            # single contiguous DMA (no 128B-element scatter)
            Msb = mres.tile([128, 1024], bf16, tag="Msb")
            nc.vector.memset(Msb[:].bitcast(f32), 0.0)

            # the dummy AR's payload is never read back, so wbin is left
            # uninitialized: the trigger then has NO data dependency and
            # fires as soon as the gpsimd queue reaches it
            wbin = dram.tile([1, 64], f32, name="wbin")
            wbo = dram.tile([1, 64], f32, name="wbo")

            # ---- phase-1 fill on TWO queues, critical bytes first. gpsimd:
            # k0/v0 weights then the dummy trigger; sync: x then k1/v1
            # weights. wq/wp defer to the scalar queue mid-kv so they don't
            # steal fill bandwidth from the kv gate. ----
            KV_SLOT = [0, 2, 1, 3]  # stream order k0, v0, k1, v1 -> kvsb col slot
            wq = [wqp.tile([128, 4096], bf16, tag=f"wq{h}", name=f"wq{h}")
                  for h in range(2)]
            wp = wpp.tile([128, 8192], bf16, tag="wp")

            # Critical 6MB (x + kv weights) spread so each stream lands just
            # before first use at ~110GB/s per queue: x alone on sync (done
            # ~9), wkv0 leading gpsimd (~13), wkv1 leading scalar (~13),
            # wkv2 split across gpsimd+sync (~13), wkv3 on scalar (~19).
            # Non-critical wq0/wp_a ride gpsimd BEHIND the dummy trigger
            # (queue blocked until the mesh begins ~t=20 - a natural time
            # gate); wq1/wp_b slot behind the critical bytes on scalar/sync.
            nc.gpsimd.dma_start(wkv[0][:, 0:1024], wkvd[0:128, 0:1024])
            nc.sync.dma_start(xsb[:, 0:1024], xd[:, 0:1024])
            nc.scalar.dma_start(xsb[:, 1024:2048], xd[:, 1024:2048])
            nc.gpsimd.dma_start(wkv[0][:, 1024:2048], wkvd[0:128, 1024:2048])
            nc.sync.dma_start(xsb[:, 2048:3072], xd[:, 2048:3072])
            nc.scalar.dma_start(xsb[:, 3072:4096], xd[:, 3072:4096])
            nc.gpsimd.dma_start(wkv[0][:, 2048:3072], wkvd[0:128, 2048:3072])
            nc.gpsimd.dma_start(wkv[0][:, 3072:4096], wkvd[0:128, 3072:4096])
            nc.scalar.dma_start(wkv[1][:, 0:2048], wkvd[128:256, 0:2048])
            nc.sync.dma_start(wkv[2][:, 0:2048], wkvd[256:384, 0:2048])
            nc.scalar.dma_start(wkv[1][:, 2048:4096], wkvd[128:256, 2048:4096])
            nc.sync.dma_start(wkv[2][:, 2048:4096], wkvd[256:384, 2048:4096])
            nc.scalar.dma_start(wkv[3][:, 0:2048], wkvd[384:512, 0:2048])
            nc.scalar.dma_start(wkv[3][:, 2048:4096], wkvd[384:512, 2048:4096])
            nc.sync.dma_start(wp[:, 4096:8192], wpd[:, 4096:8192])
            nc.sync.dma_start(wq[1][:], wqd[128:256, :])

            # dummy rendezvous: pre-pays the one-time collective setup
            nc.gpsimd.collective_compute(
                "AllReduce", mybir.AluOpType.add, replica_groups=GROUPS,
                ins=[wbin.opt()], outs=[wbo.opt()],
            )
            nc.gpsimd.dma_start(wq[0][:], wqd[0:128, :])
            nc.gpsimd.dma_start(wp[:, 0:4096], wpd[:, 0:4096])

            # warmups cover the first-chunk DMA arrival and hand the kv phase
            # a ramping PE
            psw = psM.tile([128, 512], f32, tag="mp", name="warm_ps")
            for _ in range(7):
                nc.tensor.matmul(psw[:], warm[:, 0:128], warm[:],
                                 start=True, stop=True)

            # kvsb[tt]: [128 tok, 2048] cols = [k(1024) | v(1024)] grouped feats
            kvsb = [kvp.tile([128, 2048], bf16, tag=f"kv{tt}", name=f"kv{tt}")
                    for tt in range(4)]

            # single combined AR payload (both halves, block-diagonal): the CC
            # pipeline costs ~11us PER OP, so one 256KB op beats two 128KB ops
            bin_ = dram.tile([128, 1024], bf16, name="bin")
            bo = dram.tile([128, 1024], bf16, name="bo")

            def m_half(g):
                # M blocks 4g..4g+3 (2 heads per 128-block, diagonal 64x64s)
                mp = psM.tile([128, 512], f32, tag="mp", name=f"mp{g}")
                for j in range(4):
                    blk = 4 * g + j
                    for tt in range(4):
                        nc.tensor.matmul(
                            mp[:, 128 * j:128 * (j + 1)],
                            kvsb[tt][:, 128 * blk:128 * (blk + 1)],
                            kvsb[tt][:, 1024 + 128 * blk:1024 + 128 * (blk + 1)],
                            start=(tt == 0), stop=(tt == 3),
                        )
                # diagonal 64x64 blocks -> Msb block-diagonal half (zeroed bg)
                mpv = mp[:].rearrange("p (j c) -> p j c", j=4)
                msv = Msb[:, 512 * g:512 * (g + 1)].rearrange("p (j c) -> p j c", j=4)
                nc.vector.tensor_copy(msv[0:64, :, 0:64], mpv[0:64, :, 0:64])
                nc.scalar.copy(msv[64:128, :, 64:128], mpv[64:128, :, 64:128])
                # this half's bounce rides the scalar queue (issue-only)
                nc.scalar.dma_start(bin_[:, 512 * g:512 * (g + 1)],
                                    Msb[:, 512 * g:512 * (g + 1)])
                if g == 1:
                    # ONE AllReduce for the whole M, chained warm behind the
                    # dummy on the gpsimd queue
                    nc.gpsimd.collective_compute(
                        "AllReduce", mybir.AluOpType.add, replica_groups=GROUPS,
                        ins=[bin_.opt()], outs=[bo.opt()],
                    )

            # kv quarters split into e-passes: early partials run while later
            # x/wkv chunks are still in flight
            def kv_quarter_split(s, passes):
                slot = KV_SLOT[s]
                pss = [psA.tile([128, 512], f32, tag="big", name=f"kv{s}_{tt}")
                       for tt in range(4)]
                for e_lo, e_hi in passes:
                    for tt in range(4):
                        ps = pss[tt]
                        for e in range(e_lo, e_hi):
                            nc.tensor.matmul(
                                ps[:],
                                xsb[:, 512 * e + 128 * tt:512 * e + 128 * (tt + 1)],
                                wkv[s][:, 512 * e:512 * (e + 1)],
                                start=(e == 0), stop=(e == 7),
                            )
                        if e_hi == 8:
                            evict(kvsb[tt][:, 512 * slot:512 * (slot + 1)],
                                  ps[:])

            kv_quarter_split(0, ((0, 2), (2, 4), (4, 6), (6, 8)))  # k0
            kv_quarter_split(1, ((0, 4), (4, 8)))                  # v0
            m_half(0)
            kv_quarter_split(2, ((0, 8),))                         # k1
            kv_quarter_split(3, ((0, 8),))                         # v1
            m_half(1)

            # ---- q (feature-major, [128 qf, 512 tok] per block), overlaps CC ----
            qsb = [qp.tile([128, TPC], bf16, tag=f"q{f}", name=f"q{f}")
                   for f in range(8)]
            for fq in range(8):
                wqh = wq[fq // 4]
                l = fq % 4
                ps = psA.tile([128, 512], f32, tag="big")
                for e in range(8):
                    nc.tensor.matmul(
                        ps[:],
                        wqh[:, 512 * e + 128 * l:512 * e + 128 * (l + 1)],
                        xsb[:, 512 * e:512 * (e + 1)],
                        start=(e == 0), stop=(e == 7),
                    )
                evict(qsb[fq][:], ps[:])

            # The AR result (summed M, block-diagonal) returns to SBUF as two
            # half DMAs on parallel queues; emitted here so the queues reach
            # the issue early and just wait on the AR-complete semaphore
            Mret = mres.tile([128, 1024], bf16, tag="Mret")
            nc.sync.dma_start(Mret[:, 0:512], bo[:, 0:512])
            nc.scalar.dma_start(Mret[:, 512:1024], bo[:, 512:1024])

            # ---- att blk = Msum_blk.T @ q_blk (in-place into the q tiles) ----
            for blk in range(8):
                ps = psA.tile([128, 512], f32, tag="big", name=f"att{blk}")
                nc.tensor.matmul(ps[:], Mret[:, 128 * blk:128 * (blk + 1)],
                                 qsb[blk][:], start=True, stop=True)
                evict(qsb[blk][:], ps[:])
            attsb = qsb

            # ---- out = attT.T @ wp ([512 tok, 1024 o]); evict straight to a
            # per-group buffer, stores round-robin the 3 DMA queues ----
            obuf = [stp.tile([128, 512], f32, tag=f"st{g}", name=f"st{g}")
                    for g in range(8)]
            st_q = [nc.sync, nc.gpsimd, nc.scalar]
            st_i = 0
            for oc in range(2):
                for tt in range(4):
                    g = 4 * oc + tt
                    ps = psA.tile([128, 512], f32, tag="big", name=f"o{g}")
                    for f in range(8):
                        nc.tensor.matmul(
                            ps[:],
                            attsb[f][:, 128 * tt:128 * (tt + 1)],
                            wp[:, 4096 * oc + 512 * f:4096 * oc + 512 * (f + 1)],
                            start=(f == 0), stop=(f == 7),
                        )
                    evict(obuf[g][:], ps[:])
                    st_q[st_i % 3].dma_start(
                        out[128 * tt:128 * (tt + 1), 512 * oc:512 * (oc + 1)],
                        obuf[g][:],
                    )
                    st_i += 1

    nc.compile()
    _built = nc
    return nc


LAST_RESULTS = None  # BassKernelResults of the most recent kernel() call


def _swz(a: np.ndarray) -> np.ndarray:
    """[1024, C] -> [128, 8*C]: row e*128+p -> partition p, cols e*C..e*C+C."""
    C = a.shape[1]
    return np.ascontiguousarray(
        a.reshape(8, 128, C).transpose(1, 0, 2).reshape(128, 8 * C))


def kernel(x: np.ndarray, W_qkv: np.ndarray, W_proj: np.ndarray) -> np.ndarray:
    global LAST_RESULTS
    from ml_dtypes import bfloat16
    from concourse import bass_utils

    nc = _build()

    x = np.ascontiguousarray(x, dtype=np.float32)
    W_qkv = np.ascontiguousarray(W_qkv, dtype=np.float32)
    W_proj = np.ascontiguousarray(W_proj, dtype=np.float32)

    # head-grouping permutation: grouped feature h*64+j <- original row j*16+h
    perm = np.arange(E).reshape(HD, NH).T.ravel()
    Wq_g = W_qkv[perm].astype(bfloat16)
    Wk_g = (W_qkv[E + perm] * np.float32(HD ** -0.5)).astype(bfloat16)  # exact 1/8
    Wv_g = W_qkv[2 * E + perm].astype(bfloat16)
    Wp_g = W_proj.astype(bfloat16)  # att concat order == grouped order already

    # kv stream groups k0, v0, k1, v1: each [512 kvf, 1024 xf] -> swz([1024, 512])
    kv_groups = [Wk_g[0:512], Wv_g[0:512], Wk_g[512:1024], Wv_g[512:1024]]
    wkvd_np = np.concatenate([_swz(np.ascontiguousarray(g.T)) for g in kv_groups], 0)
    wqd_np = np.concatenate(
        [_swz(np.ascontiguousarray(Wq_g[512 * h:512 * (h + 1)].T)) for h in range(2)], 0)
    # wp oc-major: [128 p(af in f), oc*4096 + f*512 + c], wp[p, ...] = Wp[o, af]
    wpT = np.ascontiguousarray(Wp_g.T)           # [1024 af, 1024 o]
    w = wpT.reshape(8, 128, 2, 512)              # [f, p, oc, c]
    wpd_np = np.ascontiguousarray(
        w.transpose(1, 2, 0, 3).reshape(128, 8192))  # [p, oc, f, c]

    in_maps = []
    for c in range(N_CORES):
        b, half = c // 2, c % 2
        xd_c = _swz(np.ascontiguousarray(
            x[b, half * TPC:(half + 1) * TPC, :].T.astype(bfloat16)))
        in_maps.append({"xd": xd_c, "wkvd": wkvd_np, "wqd": wqd_np, "wpd": wpd_np})

    import os as _os
    _tc = _os.environ.get("KERNEL_TRACE_CORES")
    _kw = {"trace_cores": [int(v) for v in _tc.split(",")]} if _tc else {}
    res = bass_utils.run_bass_kernel_spmd(nc, in_maps, core_ids=list(range(N_CORES)), **_kw)
    LAST_RESULTS = res

    out = np.empty((B, T, E), dtype=np.float32)
    for c in range(N_CORES):
        b, half = c // 2, c % 2
        out[b, half * TPC:(half + 1) * TPC, :] = res.results[c]["out"]
    return out


# revision 31
# speedup vs baseline: 1.2589x; 1.2589x over previous
"""Multi-head attention (no softmax) on 8 trn2 NeuronCores.

Reference: out = ((x @ Wqkv.T -> q,k,v per head) ; (q @ k.T * s) @ v ; concat ; @ Wproj.T)

Because there is no softmax the attention is linear:
    (q @ k.T) @ v == q @ (k.T @ v),  k.T @ v is only 64x64 per head,
so the T x T score matrices never need to exist. Per head:
    M_h = (s * k_h).T @ v_h        (64 x 64, reduced over ALL tokens of the batch)
    out += (q_h @ M_h) @ Wproj_h.T

Sharding: token-parallel. Core c owns batch b=c//2, token half c%2 (512 tokens).
M_h needs a reduction over the full batch -> pairwise AllReduce(add) between
the two cores of each batch, split into TWO 64KB ops (heads 0-7 after the
first kv half, heads 8-15 after the second) so the first returns early.

Everything runs in bf16 (same PE rate as fp32r, half the HBM traffic; rel err
~5e-3 vs the 2e-2 gate). PSUM accumulates fp32. The 1/8 head scale is folded
into W_k on the host (exact).

Schedule (the point of this revision): the PE never waits on the collective
with real work available. Order: kv half0 -> M0 -> AR0 trigger -> kv half1 ->
M1 -> AR1 trigger -> q (all 8 blocks) -> att heads0-7 (needs AR0) -> out
PHASE A: all 8 (oc,tt) groups accumulate contraction f=0..3 in PSUM and stash
the fp32 partial in SBUF -> att heads8-15 (needs AR1) -> out PHASE B:
accumulate f=4..7, add the stash back (DVE tensor_add), store. Phase A is
~8us of real matmul cover for AR1 latency/partner skew; the old 56-dummy
filler is gone. A dummy 256B AllReduce still pre-pays the one-time collective
setup during the fill (first op costs ~13us of semaphore hops).

DMAs are coarse - the host pre-swizzles every operand into its exact
[128, cols] SBUF layout so each logical group is ONE contiguous dma_start
(the Sync sequencer spends ~0.6us of issue time per dma_start). The fill is
split across two queues (sync: x + k1/v1 weights; gpsimd: k0/v0 weights
before the dummy trigger) so issue time halves and kv can start on the first
0.25MB. wq/wp (4MB) defer to the scalar queue mid-kv. Memsets ride the idle
vector engine at t=0 so warmup matmuls start immediately; output stores
round-robin 3 DMA queues.
"""

import numpy as np

B, T, E = 4, 1024, 1024
NH, HD = 16, 64
N_CORES = 8
TPC = T // 2  # tokens per core = 512

_built = None


def _build():
    """Build + compile the 8-core SPMD Bass program once."""
    global _built
    if _built is not None:
        return _built

    import concourse.mybir as mybir
    import concourse.tile as tile
    from concourse import bacc

    f32 = mybir.dt.float32
    bf16 = mybir.dt.bfloat16
    GROUPS = [[0, 1], [2, 3], [4, 5], [6, 7]]

    nc = bacc.Bacc("TRN2", target_bir_lowering=False, debug=False, num_devices=N_CORES)
    # x pre-swizzled: [128 part, e*512 + tok]
    xd = nc.dram_tensor("xd", [128, 4096], bf16, kind="ExternalInput").ap()
    # kv weights: 4 stream groups (k0, v0, k1, v1), each [128 part, e*512 + kvf]
    wkvd = nc.dram_tensor("wkvd", [4 * 128, 8 * 512], bf16, kind="ExternalInput").ap()
    # q weights: 2 column-half groups, each [128 part, e*512 + qf]
    wqd = nc.dram_tensor("wqd", [2 * 128, 8 * 512], bf16, kind="ExternalInput").ap()
    # proj weights, oc-major: [128 part, oc*4096 + f*512 + c]
    wpd = nc.dram_tensor("wpd", [128, 8 * 1024], bf16, kind="ExternalInput").ap()
    out = nc.dram_tensor("out", [TPC, E], f32, kind="ExternalOutput").ap()

    evict_i = [0]

    def evict(dst, src):
        # spread PSUM->SBUF eviction copies across DVE and ACT
        if evict_i[0] % 2 == 0:
            nc.vector.tensor_copy(dst, src)
        else:
            nc.scalar.copy(dst, src)
        evict_i[0] += 1

    with tile.TileContext(nc) as tc:
        with (
            tc.tile_pool(name="xp", bufs=1) as xp,
            tc.tile_pool(name="wkvp", bufs=1) as wkvp,
            tc.tile_pool(name="kvp", bufs=1) as kvp,
            tc.tile_pool(name="wqp", bufs=1) as wqp,
            tc.tile_pool(name="wpp", bufs=1) as wpp,
            tc.tile_pool(name="qp", bufs=1) as qp,
            tc.tile_pool(name="mres", bufs=1) as mres,
            tc.tile_pool(name="stp", bufs=1) as stp,
            tc.tile_pool(name="warm", bufs=1) as warmp,
            tc.tile_pool(name="dram", bufs=1, space="DRAM") as dram,
            tc.tile_pool(name="psA", bufs=6, space="PSUM") as psA,
            tc.tile_pool(name="psM", bufs=2, space="PSUM") as psM,
        ):
            xsb = xp.tile([128, 4096], bf16, tag="x")  # col = e*512 + tok
            wkvt = wkvp.tile([128, 16384], bf16, tag="wkv")
            wkv = [wkvt[:, 4096 * s:4096 * (s + 1)] for s in range(4)]

            # ---- t=0: memsets on the idle DVE so the PE warms immediately ----
            warm = warmp.tile([128, 512], bf16, tag="warm")
            nc.vector.memset(warm[:].bitcast(f32), 0.0)
            # Msb holds both AR payloads in block-diagonal layout; the
            # off-diagonal zeros ride along so the AR result can return as a
            # single contiguous DMA (no 128B-element scatter)
            Msb = mres.tile([128, 1024], bf16, tag="Msb")
            nc.vector.memset(Msb[:].bitcast(f32), 0.0)

            # the dummy AR's payload is never read back, so wbin is left
            # uninitialized: the trigger then has NO data dependency and
            # fires as soon as the gpsimd queue reaches it
            wbin = dram.tile([1, 64], f32, name="wbin")
            wbo = dram.tile([1, 64], f32, name="wbo")

            # ---- phase-1 fill on TWO queues, critical bytes first. gpsimd:
            # k0/v0 weights then the dummy trigger; sync: x then k1/v1
            # weights. wq/wp defer to the scalar queue mid-kv so they don't
            # steal fill bandwidth from the kv gate. ----
            KV_SLOT = [0, 2, 1, 3]  # stream order k0, v0, k1, v1 -> kvsb col slot
            wqt = wqp.tile([128, 8192], bf16, tag="wq")
            wp = wpp.tile([128, 8192], bf16, tag="wp")

            # Critical 6MB (x + kv weights) spread so each stream lands just
            # before first use at ~110GB/s per queue: x alone on sync (done
            # ~9), wkv0 leading gpsimd (~13), wkv1 leading scalar (~13),
            # wkv2 split across gpsimd+sync (~13), wkv3 on scalar (~19).
            # Non-critical wq0/wp_a ride gpsimd BEHIND the dummy trigger
            # (queue blocked until the mesh begins ~t=20 - a natural time
            # gate); wq1/wp_b slot behind the critical bytes on scalar/sync.
            nc.gpsimd.dma_start(wkvt[:, 0:1024], wkvd[0:128, 0:1024])
            nc.sync.dma_start(xsb[:, 0:1024], xd[:, 0:1024])
            nc.scalar.dma_start(xsb[:, 1024:2048], xd[:, 1024:2048])
            nc.gpsimd.dma_start(wkvt[:, 1024:2048], wkvd[0:128, 1024:2048])
            nc.sync.dma_start(xsb[:, 2048:3072], xd[:, 2048:3072])
            nc.scalar.dma_start(xsb[:, 3072:4096], xd[:, 3072:4096])
            nc.gpsimd.dma_start(wkvt[:, 2048:3072], wkvd[0:128, 2048:3072])
            nc.gpsimd.dma_start(wkvt[:, 3072:4096], wkvd[0:128, 3072:4096])
            nc.scalar.dma_start(wkvt[:, 4096:6144], wkvd[128:256, 0:2048])
            nc.sync.dma_start(wkvt[:, 8192:10240], wkvd[256:384, 0:2048])
            nc.scalar.dma_start(wkvt[:, 6144:8192], wkvd[128:256, 2048:4096])
            nc.sync.dma_start(wkvt[:, 10240:12288], wkvd[256:384, 2048:4096])
            nc.scalar.dma_start(wkvt[:, 12288:14336], wkvd[384:512, 0:2048])
            nc.scalar.dma_start(wkvt[:, 14336:16384], wkvd[384:512, 2048:4096])
            nc.sync.dma_start(wp[:, 4096:8192], wpd[:, 4096:8192])
            nc.sync.dma_start(wqt[:, 4096:8192], wqd[128:256, :])

            # dummy rendezvous: pre-pays the one-time collective setup
            nc.gpsimd.collective_compute(
                "AllReduce", mybir.AluOpType.add, replica_groups=GROUPS,
                ins=[wbin.opt()], outs=[wbo.opt()],
            )
            nc.gpsimd.dma_start(wqt[:, 0:4096], wqd[0:128, :])
            nc.gpsimd.dma_start(wp[:, 0:4096], wpd[:, 0:4096])

            # warmups cover the first-chunk DMA arrival and hand the kv phase
            # a ramping PE
            psw = psM.tile([128, 512], f32, tag="mp", name="warm_ps")
            for _ in range(7):
                nc.tensor.matmul(psw[:], warm[:, 0:128], warm[:],
                                 start=True, stop=True)

            # kvsb[tt]: [128 tok, 2048] cols = [k(1024) | v(1024)] grouped feats
            kvsb = [kvp.tile([128, 2048], bf16, tag=f"kv{tt}", name=f"kv{tt}")
                    for tt in range(4)]

            # single combined AR payload (both halves, block-diagonal): the CC
            # pipeline costs ~11us PER OP, so one 256KB op beats two 128KB ops
            bin_ = dram.tile([128, 1024], bf16, name="bin")
            bo = dram.tile([128, 1024], bf16, name="bo")

            def m_half(g):
                # M blocks 4g..4g+3 (2 heads per 128-block, diagonal 64x64s)
                mp = psM.tile([128, 512], f32, tag="mp", name=f"mp{g}")
                for j in range(4):
                    blk = 4 * g + j
                    for tt in range(4):
                        nc.tensor.matmul(
                            mp[:, 128 * j:128 * (j + 1)],
                            kvsb[tt][:, 128 * blk:128 * (blk + 1)],
                            kvsb[tt][:, 1024 + 128 * blk:1024 + 128 * (blk + 1)],
                            start=(tt == 0), stop=(tt == 3),
                        )
                # diagonal 64x64 blocks -> Msb block-diagonal half (zeroed bg)
                mpv = mp[:].rearrange("p (j c) -> p j c", j=4)
                msv = Msb[:, 512 * g:512 * (g + 1)].rearrange("p (j c) -> p j c", j=4)
                nc.vector.tensor_copy(msv[0:64, :, 0:64], mpv[0:64, :, 0:64])
                nc.scalar.copy(msv[64:128, :, 64:128], mpv[64:128, :, 64:128])
                # this half's bounce rides the scalar queue (issue-only)
                nc.scalar.dma_start(bin_[:, 512 * g:512 * (g + 1)],
                                    Msb[:, 512 * g:512 * (g + 1)])
                if g == 1:
                    # ONE AllReduce for the whole M, chained warm behind the
                    # dummy on the gpsimd queue
                    nc.gpsimd.collective_compute(
                        "AllReduce", mybir.AluOpType.add, replica_groups=GROUPS,
                        ins=[bin_.opt()], outs=[bo.opt()],
                    )

            # kv quarters split into e-passes: early partials run while later
            # x/wkv chunks are still in flight
            def kv_quarter_split(s, passes):
                slot = KV_SLOT[s]
                pss = [psA.tile([128, 512], f32, tag="big", name=f"kv{s}_{tt}")
                       for tt in range(4)]
                for e_lo, e_hi in passes:
                    for tt in range(4):
                        ps = pss[tt]
                        for e in range(e_lo, e_hi):
                            nc.tensor.matmul(
                                ps[:],
                                xsb[:, 512 * e + 128 * tt:512 * e + 128 * (tt + 1)],
                                wkvt[:, 4096 * s + 512 * e:4096 * s + 512 * (e + 1)],
                                start=(e == 0), stop=(e == 7),
                            )
                        if e_hi == 8:
                            evict(kvsb[tt][:, 512 * slot:512 * (slot + 1)],
                                  ps[:])

            kv_quarter_split(0, ((0, 2), (2, 4), (4, 6), (6, 8)))  # k0
            kv_quarter_split(1, ((0, 4), (4, 8)))                  # v0
            m_half(0)
            kv_quarter_split(2, ((0, 8),))                         # k1
            kv_quarter_split(3, ((0, 8),))                         # v1
            m_half(1)

            # ---- q (feature-major, [128 qf, 512 tok] per block), overlaps CC ----
            qsb = [qp.tile([128, TPC], bf16, tag=f"q{f}", name=f"q{f}")
                   for f in range(8)]
            for fq in range(8):
                l = fq % 4
                ps = psA.tile([128, 512], f32, tag="big")
                for e in range(8):
                    nc.tensor.matmul(
                        ps[:],
                        wqt[:, 4096 * (fq // 4) + 512 * e + 128 * l:
                             4096 * (fq // 4) + 512 * e + 128 * (l + 1)],
                        xsb[:, 512 * e:512 * (e + 1)],
                        start=(e == 0), stop=(e == 7),
                    )
                evict(qsb[fq][:], ps[:])

            # The AR result (summed M, block-diagonal) returns to SBUF as two
            # half DMAs on parallel queues; emitted here so the queues reach
            # the issue early and just wait on the AR-complete semaphore
            Mret = mres.tile([128, 1024], bf16, tag="Mret")
            nc.sync.dma_start(Mret[:, 0:512], bo[:, 0:512])
            nc.scalar.dma_start(Mret[:, 512:1024], bo[:, 512:1024])

            # ---- att blk = Msum_blk.T @ q_blk (in-place into the q tiles) ----
            for blk in range(8):
                ps = psA.tile([128, 512], f32, tag="big", name=f"att{blk}")
                nc.tensor.matmul(ps[:], Mret[:, 128 * blk:128 * (blk + 1)],
                                 qsb[blk][:], start=True, stop=True)
                evict(qsb[blk][:], ps[:])
            attsb = qsb

            # ---- out = attT.T @ wp ([512 tok, 1024 o]); evict straight to a
            # per-group buffer, stores round-robin the 3 DMA queues ----
            obuf = [stp.tile([128, 512], f32, tag=f"st{g}", name=f"st{g}")
                    for g in range(8)]
            st_q = [nc.sync, nc.gpsimd, nc.scalar]
            st_i = 0
            for oc in range(2):
                for tt in range(4):
                    g = 4 * oc + tt
                    ps = psA.tile([128, 512], f32, tag="big", name=f"o{g}")
                    for f in range(8):
                        nc.tensor.matmul(
                            ps[:],
                            attsb[f][:, 128 * tt:128 * (tt + 1)],
                            wp[:, 4096 * oc + 512 * f:4096 * oc + 512 * (f + 1)],
                            start=(f == 0), stop=(f == 7),
                        )
                    evict(obuf[g][:], ps[:])
                    # the last two groups split their stores across two queues
                    # each, so the final drain waits on ~128KB, not 256KB
                    if g >= 6:
                        for h in range(2):
                            st_q[st_i % 3].dma_start(
                                out[128 * tt:128 * (tt + 1),
                                    512 * oc + 256 * h:512 * oc + 256 * (h + 1)],
                                obuf[g][:, 256 * h:256 * (h + 1)],
                            )
                            st_i += 1
                    else:
                        st_q[st_i % 3].dma_start(
                            out[128 * tt:128 * (tt + 1), 512 * oc:512 * (oc + 1)],
                            obuf[g][:],
                        )
                        st_i += 1

    nc.compile()
    _built = nc
    return nc


LAST_RESULTS = None  # BassKernelResults of the most recent kernel() call


def _swz(a: np.ndarray) -> np.ndarray:
    """[1024, C] -> [128, 8*C]: row e*128+p -> partition p, cols e*C..e*C+C."""
    C = a.shape[1]
    return np.ascontiguousarray(
        a.reshape(8, 128, C).transpose(1, 0, 2).reshape(128, 8 * C))


def kernel(x: np.ndarray, W_qkv: np.ndarray, W_proj: np.ndarray) -> np.ndarray:
    global LAST_RESULTS
    from ml_dtypes import bfloat16
    from concourse import bass_utils

    nc = _build()

    x = np.ascontiguousarray(x, dtype=np.float32)
    W_qkv = np.ascontiguousarray(W_qkv, dtype=np.float32)
    W_proj = np.ascontiguousarray(W_proj, dtype=np.float32)

    # head-grouping permutation: grouped feature h*64+j <- original row j*16+h
    perm = np.arange(E).reshape(HD, NH).T.ravel()
    Wq_g = W_qkv[perm].astype(bfloat16)
    Wk_g = (W_qkv[E + perm] * np.float32(HD ** -0.5)).astype(bfloat16)  # exact 1/8
    Wv_g = W_qkv[2 * E + perm].astype(bfloat16)
    Wp_g = W_proj.astype(bfloat16)  # att concat order == grouped order already

    # kv stream groups k0, v0, k1, v1: each [512 kvf, 1024 xf] -> swz([1024, 512])
    kv_groups = [Wk_g[0:512], Wv_g[0:512], Wk_g[512:1024], Wv_g[512:1024]]
    wkvd_np = np.concatenate([_swz(np.ascontiguousarray(g.T)) for g in kv_groups], 0)
    wqd_np = np.concatenate(
        [_swz(np.ascontiguousarray(Wq_g[512 * h:512 * (h + 1)].T)) for h in range(2)], 0)
    # wp oc-major: [128 p(af in f), oc*4096 + f*512 + c], wp[p, ...] = Wp[o, af]
    wpT = np.ascontiguousarray(Wp_g.T)           # [1024 af, 1024 o]
    w = wpT.reshape(8, 128, 2, 512)              # [f, p, oc, c]
    wpd_np = np.ascontiguousarray(
        w.transpose(1, 2, 0, 3).reshape(128, 8192))  # [p, oc, f, c]

    in_maps = []
    for c in range(N_CORES):
        b, half = c // 2, c % 2
        xd_c = _swz(np.ascontiguousarray(
            x[b, half * TPC:(half + 1) * TPC, :].T.astype(bfloat16)))
        in_maps.append({"xd": xd_c, "wkvd": wkvd_np, "wqd": wqd_np, "wpd": wpd_np})

    import os as _os
    _tc = _os.environ.get("KERNEL_TRACE_CORES")
    _kw = {"trace_cores": [int(v) for v in _tc.split(",")]} if _tc else {}
    res = bass_utils.run_bass_kernel_spmd(nc, in_maps, core_ids=list(range(N_CORES)), **_kw)
    LAST_RESULTS = res

    out = np.empty((B, T, E), dtype=np.float32)
    for c in range(N_CORES):
        b, half = c // 2, c % 2
        out[b, half * TPC:(half + 1) * TPC, :] = res.results[c]["out"]
    return out
